# revision 8
# baseline (speedup 1.0000x reference)
"""Nearest-E8-lattice quantizer (CachedE8Quantizer) as a Bass/Tile kernel on 8 trn2 cores.

Input x: [8388608, 8] fp32. Output: nearest point of E8 = D8 u (D8 + 1/2).

Sharding: data-parallel over the points dim, 1/8 per core (no comms).

Math (per 8-vector):
  r  = round(x) (RNE)            d = x - r in [-.5,.5]
  h  = copysign(.5, d)           a = |d|
  Branch-1 rounding r1h = r + h exactly (nearest half-odd-integer grid), and
  |d1| = .5 - a, sum d1^2 = sum d^2 - sum a + 2.  So with
    p0 = parity(sum r),  p1 = parity(sum r + sum h)  (sum h integer-valued),
    m0 = max a, n0 = min a, A = sum a,
  branch choice c = [ p0*(1-2*m0) - 2*p1*n0 + A <= 2 ]  (D0 <= D1 in A-form).
  Output y = r + h*G,  G = e*(4c-2) + (1-c),  e = [a == t],
    t = c ? (p0 ? m0 : -1) : (p1 ? n0 : -1)   (-1 never matches a >= 0).

Engines: PE computes all segmented sums (8 accumulating stride-8 identity
matmuls each) and the final y = r + h*G sum; ACT does |d|, parity squares and
PSUM->SBUF copies; DVE/GPSIMD split the remaining elementwise/reduce work.
"""

import numpy as np

from concourse import bacc
import concourse.mybir as mybir
from concourse.alu_op_type import AluOpType as op
from concourse.bass_utils import run_bass_kernel_spmd
from concourse.tile import TileContext

N_POINTS = 8388608
N_CORES = 8
SHARD = N_POINTS // N_CORES  # 1048576 points per core

MAGIC = 12582912.0  # 1.5 * 2**23: (x + MAGIC) - MAGIC == round-half-even(x)
F32 = mybir.dt.float32
U32 = mybir.dt.uint32
X = mybir.AxisListType.X
CP = mybir.ActivationFunctionType.Copy

# engine assignment per op (tunable): "vector" (DVE) or "gpsimd" (Pool).
# GP (Pool) legality: TT {add,subtract,mult} incl. broadcast/3D views, TS (any
# scalar ops), copy.  DVE-only: is_equal, STT, X-axis tensor_reduce,
# copy_predicated.
ENG = {
    "r": "vector",      # round TS (2x on DVE)
    "d": "gpsimd",      # x - r TT (sub: GP-legal)
    "h": "vector",      # copysign TS (2x on DVE)
    "maxa": "vector",   # segmented max reduce (X-axis reduce is DVE-only)
    "mina": "vector",   # segmented min reduce (X-axis reduce is DVE-only)
    "e": "vector",      # onehot is_equal TT (DVE-only)
    "gm": "gpsimd",     # G = e*scale2_b (mult: GP-legal)
    "ga": "gpsimd",     # G += oc_b (add: GP-legal)
    "hg": "vector",     # hg = h*G
    # smalls
    "sf1": "gpsimd",
    "k1": "vector",
    "k2": "vector",
    "ps": "vector",     # STT: DVE-only
    "v": "gpsimd",
    "g1": "gpsimd",
    "g2": "gpsimd",
    "g3": "vector",     # STT: DVE-only
    "c": "gpsimd",
    "oc": "gpsimd",
    "w2": "gpsimd",
    "tt2": "gpsimd",
    "w2m": "vector",
    "t2": "gpsimd",
    "tcopy": "vector",
    "scale2": "vector",
}


def _emit_tile(nc, pools, xd, yd, t, tf):
    E = lambda k: getattr(nc, ENG[k])
    P = 128
    pts = P * tf
    W = tf * 8          # elems per partition per tile
    S = tf              # segments (points) per partition per tile
    stream, work, small, psum_pool, ident = pools

    s = t * pts
    x_rows = xd[s : s + pts, :].rearrange("(p f) c -> p (f c)", p=P)
    y_rows = yd[s : s + pts, :].rearrange("(p f) c -> p (f c)", p=P)

    # stream slot lifecycle: xt -> d (in-place) -> yt
    xt = stream.tile([P, W], F32, tag="xt")
    nc.sync.dma_start(out=xt[:], in_=x_rows)

    rt = work.tile([P, W], F32, tag="rt")
    ht = work.tile([P, W], F32, tag="ht")
    at = work.tile([P, W], F32, tag="at")

    # r = round(x);  d = x - r (in-place over xt);  a = |d|;  h = copysign(.5, d)
    E("r").tensor_scalar(rt[:], xt[:], MAGIC, MAGIC, op0=op.add, op1=op.subtract)
    dt = xt  # alias: d overwrites the stream slot
    E("d").tensor_tensor(dt[:], xt[:], rt[:], op.subtract)
    nc.scalar.activation(at[:], dt[:], mybir.ActivationFunctionType.Abs)
    E("h").tensor_scalar(ht[:], dt[:], 0.0, 0.5, op0=op.is_ge, op1=op.subtract)

    # arena layout (units of S): 0 SR | 1 SF1 | 2 SA | 3 m0 | 4 n0 | 5:7 K |
    # 7:9 P2 | 9 Z0 | 10 Z1 | 11 c | 12 oc | 13 t
    ar = small.tile([P, 14 * S], F32, tag="ar")
    SR = ar[:, 0 * S : 1 * S]
    SF1 = ar[:, 1 * S : 2 * S]
    SA = ar[:, 2 * S : 3 * S]
    M2 = ar[:, 3 * S : 5 * S]
    K2 = ar[:, 5 * S : 7 * S]
    PK = ar[:, 0 * S : 2 * S]  # (SR | SF1)
    P2 = ar[:, 7 * S : 9 * S]
    Z0 = ar[:, 9 * S : 10 * S]
    Z1 = ar[:, 10 * S : 11 * S]
    cS = ar[:, 11 * S : 12 * S]
    C2 = ar[:, 11 * S : 13 * S]  # (c | oc)
    tS = ar[:, 13 * S : 14 * S]

    # PE segmented sums: for src in (r, h, a): psum[:, j] = sum_c src[p, 8j+c]
    for k, (src, dst) in enumerate(((rt, SR), (ht, SF1), (at, SA))):
        ps = psum_pool.tile([P, S], F32, tag=f"ps{k}", bufs=2)
        src3 = src[:].rearrange("p (t c) -> p t c", c=8)
        for c in range(8):
            nc.tensor.matmul(ps[:], ident[:], src3[:, :, c], start=(c == 0), stop=(c == 7))
        nc.scalar.copy(dst, ps[:])

    # segmented max/min of a
    at3 = at[:].rearrange("p (t c) -> p t c", c=8)
    E("maxa").tensor_reduce(M2[:, 0:S], at3, axis=X, op=op.max)
    E("mina").tensor_reduce(M2[:, S : 2 * S], at3, axis=X, op=op.min)

    # SF1 currently holds sum(h); SF1 = SR + sum(h)  (= sum f1 + 4, integer)
    E("sf1").tensor_tensor(SF1, SR, SF1, op.add)

    # parity of PK = (SR | SF1): ps = 2*round(s/2) - s in {-1,0,1}; P2 = ps^2
    E("k1").tensor_scalar(K2[:], PK, 0.5, MAGIC, op0=op.mult, op1=op.add)
    E("k2").tensor_scalar(K2[:], K2[:], MAGIC, None, op0=op.subtract)
    E("ps").scalar_tensor_tensor(K2[:], K2[:], 2.0, PK, op0=op.mult, op1=op.subtract)
    nc.scalar.square(P2[:], K2[:])

    # c = [p0*(1-2m0) - 2*p1*n0 + A <= 2]  <=>  [SA + p0 - 2*(p0*m0 + p1*n0) <= 2]
    V2 = K2  # reuse K slots (ps dead)
    E("v").tensor_tensor(V2[:], P2[:], M2[:], op.mult)
    E("g1").tensor_tensor(Z0, V2[:, 0:S], V2[:, S : 2 * S], op.add)
    E("g2").tensor_tensor(Z1, SA, P2[:, 0:S], op.add)
    E("g3").scalar_tensor_tensor(Z0, Z0, -2.0, Z1, op0=op.mult, op1=op.add)
    E("c").tensor_scalar(cS, Z0, 2.0, None, op0=op.is_le)
    E("oc").tensor_scalar(C2[:, S : 2 * S], Z0, 2.0, None, op0=op.is_gt)

    # gated compare target t = c ? (p0 ? m0 : -1) : (p1 ? n0 : -1)
    W2 = K2  # reuse again (V dead)
    E("w2").tensor_tensor(W2[:], P2[:], C2[:], op.mult)
    TT2 = P2  # reuse (P2 dead after W2... p0 still needed? no: g2 consumed it)
    E("tt2").tensor_tensor(TT2[:], M2[:], W2[:], op.mult)
    E("w2m").tensor_scalar(W2[:], W2[:], 1.0, None, op0=op.subtract)
    E("t2").tensor_tensor(TT2[:], TT2[:], W2[:], op.add)  # (t0 | t1)
    E("tcopy").tensor_copy(tS, TT2[:, S : 2 * S])
    nc.vector.copy_predicated(tS, cS.bitcast(U32), TT2[:, 0:S])
    E("scale2").tensor_scalar(cS, cS, 4.0, 2.0, op0=op.mult, op1=op.subtract)
    # cS now holds scale2 = 4c-2; oc slot holds (1-c); both exact {.,.}

    # e = [a == t]; G = e*scale2 + oc; hg = h*G
    t_b = tS.unsqueeze(2).broadcast_to([P, S, 8])
    sc_b = cS.unsqueeze(2).broadcast_to([P, S, 8])
    oc_b = C2[:, S : 2 * S].unsqueeze(2).broadcast_to([P, S, 8])
    et = at  # e overwrites a (elementwise in-place)
    et3 = et[:].rearrange("p (t c) -> p t c", c=8)
    E("e").tensor_tensor(et3, at3, t_b, op.is_equal)
    E("gm").tensor_tensor(et3, et3, sc_b, op.mult)
    E("ga").tensor_tensor(et3, et3, oc_b, op.add)
    hgt = ht  # hg overwrites h
    E("hg").tensor_tensor(hgt[:], ht[:], et[:], op.mult)

    # y = r + hg via PE accumulate, 512-wide psum chunks; copy out; DMA
    yt = xt  # reuse stream slot (d dead)
    NCH = 512
    for c0 in range(0, W, NCH):
        yp = psum_pool.tile([P, NCH], F32, tag="yp", bufs=2)
        sl = slice(c0, c0 + NCH)
        nc.tensor.matmul(yp[:], ident[:], rt[:, sl], start=True, stop=False)
        nc.tensor.matmul(yp[:], ident[:], hgt[:, sl], start=False, stop=True)
        nc.scalar.copy(yt[:, sl], yp[:])
    nc.sync.dma_start(out=y_rows, in_=yt[:])


def build_nc(shard=SHARD, tf=512, reps=1):
    P = 128
    pts = P * tf
    assert shard % pts == 0
    ntiles = shard // pts

    nc = bacc.Bacc("TRN2", target_bir_lowering=False, debug=False, num_devices=N_CORES)
    xd = nc.declare_dram_parameter("x", [shard, 8], F32, isOutput=False)
    yd = nc.declare_dram_parameter("y", [shard, 8], F32, isOutput=True)

    from concourse.masks import make_identity

    with TileContext(nc) as tc:
        with (
            tc.tile_pool(name="stream", bufs=2) as stream,
            tc.tile_pool(name="work", bufs=2) as work,
            tc.tile_pool(name="small", bufs=2) as small,
            tc.tile_pool(name="const", bufs=1) as cpool,
            tc.tile_pool(name="psum", bufs=2, space="PSUM") as psum_pool,
        ):
            ident = cpool.tile([P, P], F32, tag="ident")
            make_identity(nc, ident[:])
            pools = (stream, work, small, psum_pool, ident)
            for _ in range(reps):
                for t in range(ntiles):
                    _emit_tile(nc, pools, xd, yd, t, tf)
    nc.finalize()
    return nc


_BUILD_CACHE = {}


def _get_nc(shard, tf):
    key = (shard, tf)
    if key not in _BUILD_CACHE:
        _BUILD_CACHE[key] = build_nc(shard, tf)
    return _BUILD_CACHE[key]


def kernel(x: np.ndarray) -> np.ndarray:
    x = np.ascontiguousarray(x, dtype=np.float32)
    n = x.shape[0]
    shard = n // N_CORES
    tf = 512
    while shard % (128 * tf) != 0:
        tf //= 2
    nc = _get_nc(shard, tf)
    in_maps = [{"x": x[i * shard : (i + 1) * shard]} for i in range(N_CORES)]
    res = run_bass_kernel_spmd(nc, in_maps, list(range(N_CORES))).results
    return np.concatenate([res[i]["y"] for i in range(N_CORES)], axis=0)
